# revision 1
# baseline (speedup 1.0000x reference)
"""Tensor-parallel GQA multi-head-attention kernel for 8 trn2 NeuronCores.

Problem: B=2, T=2048, D=2048, H=16 q-heads, KV=4 kv-heads, HD=128,
causal attention with interleaved RoPE, y = attn_out @ Wo.

Sharding (tensor-parallel over heads, per the hint):
  core c = b*4 + g   (b = batch index, g = kv-head / q-head-group index)
  Each core computes q-heads 4g..4g+3 and kv-head g for batch b, plus the
  partial output  y_partial = attn_heads @ Wo[rows of those heads]  (row-
  parallel Wo).  The host sums the 4 partials per batch (the unshard of the
  row-parallel all-reduce) and stacks the 2 batches.

On-chip design (per core, everything bf16 except PSUM/softmax math):
  - host pre-transposes x -> xT [D,T] and permutes Wq/Wk columns per head to
    [even dims | odd dims] so RoPE pairs live in partition halves.
  - projections: q^T[h] = Wq_h^T @ xT  (lhsT=Wq chunk), k^T likewise,
    v natural via lhsT = xT block.
  - RoPE: rot = q*cos_dup + swap(q)*[-sin|sin]; the half-swap is an
    SBUF->SBUF DMA, the rest DVE.
  - attention per (head, 512-wide q chunk): for each 128-row k tile
    S^T = k^T_tile.T(dot) q^T chunk -> PSUM [128,512]; diagonal blocks get a
    -30000 mask add (DVE); ACT computes P = exp(scale*S^T) -> SBUF bf16;
    PV accumulates out^T[HD,512] with lhsT = v tile; an all-ones [128,128]
    lhsT matmul accumulates the softmax denominators broadcast across all
    128 partitions; normalization = reciprocal + one DVE multiply.
    Fully-masked (future) blocks are skipped -> ~40% less attention work.
  - Wo: y tile [128,512] = sum_h attnT_h chunk.T @ Wo_h chunk, DVE copy to
    SBUF, DMA to DRAM.
"""

import math
import sys

import numpy as np

for _p in ("/opt/trn_rl_repo", "/root/.axon_site",
           "/root/.axon_site/_ro/trn_rl_repo",
           "/root/.axon_site/_ro/pypackages"):
    if _p not in sys.path:
        sys.path.append(_p)

B, T, D = 2, 2048, 2048
H, KV, HD = 16, 4, 128
ROPE_BASE = 10000.0
N_CORES = 8
HEADS_PER_CORE = H // KV // (N_CORES // (B * KV)) if False else 4  # 4
DQ = HEADS_PER_CORE * HD  # 512 q-dims per core
SCALE = 1.0 / math.sqrt(HD)
MASK_VAL = -30000.0

_CACHE = {}


def _build_nc(t_len=T):
    """Build the single-core SPMD Bass/Tile program (cached)."""
    import concourse.bass as bass
    import concourse.mybir as mybir
    import concourse.tile as tile
    from concourse import bacc

    f32 = mybir.dt.float32
    bf16 = mybir.dt.bfloat16
    ts = bass.ts

    NT = t_len // 128        # number of 128-row T tiles
    NK = D // 128            # contraction chunks for projections
    NCQ = t_len // 512       # number of 512-wide q chunks

    nc = bacc.Bacc("TRN2", target_bir_lowering=False, debug=False,
                   num_devices=N_CORES)

    xT_d = nc.dram_tensor("xT", [D, t_len], bf16, kind="ExternalInput").ap()
    wq_d = nc.dram_tensor("wq", [D, DQ], bf16, kind="ExternalInput").ap()
    wk_d = nc.dram_tensor("wk", [D, HD], bf16, kind="ExternalInput").ap()
    wv_d = nc.dram_tensor("wv", [D, HD], bf16, kind="ExternalInput").ap()
    wo_d = nc.dram_tensor("wo", [DQ, D], bf16, kind="ExternalInput").ap()
    cos_d = nc.dram_tensor("cosd", [128, t_len], bf16, kind="ExternalInput").ap()
    ssig_d = nc.dram_tensor("ssig", [128, t_len], bf16, kind="ExternalInput").ap()
    mask_d = nc.dram_tensor("mask", [128, 128], bf16, kind="ExternalInput").ap()
    y_d = nc.dram_tensor("y", [t_len, D], f32, kind="ExternalOutput").ap()

    Exp = mybir.ActivationFunctionType.Exp

    with tile.TileContext(nc) as tc:
        with (
            tc.tile_pool(name="const", bufs=1) as const,
            tc.tile_pool(name="qkv", bufs=1) as qkv,
            tc.tile_pool(name="attn", bufs=3) as attn_pool,
            tc.tile_pool(name="p", bufs=6) as p_pool,
            tc.tile_pool(name="rope", bufs=2) as rope_pool,
            tc.tile_pool(name="recip", bufs=2) as recip_pool,
            tc.tile_pool(name="y", bufs=3) as y_pool,
            tc.tile_pool(name="psum", bufs=1, space="PSUM") as psum,
        ):
            # ---- constant / input loads (per-k-chunk tiles so compute can
            # start as soon as the first chunks land) ----
            xT = [const.tile([128, t_len], bf16, tag=f"xT{k}", name=f"xT{k}") for k in range(NK)]
            wq = [const.tile([128, DQ], bf16, tag=f"wq{k}", name=f"wq{k}") for k in range(NK)]
            wk = [const.tile([128, HD], bf16, tag=f"wk{k}", name=f"wk{k}") for k in range(NK)]
            wv = [const.tile([128, HD], bf16, tag=f"wv{k}", name=f"wv{k}") for k in range(NK)]
            # rope tables + mask for chunk 0 first: they gate the rope ->
            # attention chain that provides the DMA-free PE work which
            # fills the input-load window
            cos_sb = const.tile([128, t_len], bf16, tag="cos")
            ssig_sb = const.tile([128, t_len], bf16, tag="ssig")
            mask_sb = const.tile([128, 128], bf16, tag="mask")
            nc.sync.dma_start(cos_sb[:, 0:512], cos_d[:, 0:512])
            nc.sync.dma_start(ssig_sb[:, 0:512], ssig_d[:, 0:512])
            nc.sync.dma_start(mask_sb[:], mask_d[:])
            # chunk-major loads: compute on chunk 0 starts after ~2MB lands
            for k in range(NK):
                nc.sync.dma_start(xT[k][:, 0:512], xT_d[ts(k, 128), 0:512])
                nc.sync.dma_start(wq[k][:], wq_d[ts(k, 128), :])
                nc.sync.dma_start(wk[k][:], wk_d[ts(k, 128), :])
                nc.sync.dma_start(wv[k][:], wv_d[ts(k, 128), :])
            wo = [const.tile([128, D], bf16, tag=f"wo{h}", name=f"wo{h}") for h in range(4)]
            for h in range(4):
                nc.sync.dma_start(wo[h][:], wo_d[ts(h, 128), :])
            for c in range(1, NCQ):
                for k in range(NK):
                    nc.sync.dma_start(xT[k][:, ts(c, 512)], xT_d[ts(k, 128), ts(c, 512)])
                nc.sync.dma_start(cos_sb[:, ts(c, 512)], cos_d[:, ts(c, 512)])
                nc.sync.dma_start(ssig_sb[:, ts(c, 512)], ssig_d[:, ts(c, 512)])
            ones_sb = const.tile([128, 128], bf16, tag="ones")
            nc.vector.memset(ones_sb[:], 1.0)

            # persistent activations
            qT = qkv.tile([128, HEADS_PER_CORE, t_len], bf16, tag="qT")
            kT = qkv.tile([128, t_len], bf16, tag="kT")
            v_sb = qkv.tile([128, NT, HD], bf16, tag="v")

            def rope_to(dst_ap, psum_tile, c):
                """Apply RoPE to a [128, 512] psum tile (rows = [even|odd]
                dims of one head, cols = T positions of chunk c); write bf16
                to dst_ap."""
                cs = slice(c * 512, (c + 1) * 512)
                qf = rope_pool.tile([128, 512], f32, tag="qf")
                nc.vector.tensor_copy(qf[:], psum_tile[:])
                qs = rope_pool.tile([128, 512], f32, tag="qs")
                nc.gpsimd.dma_start(qs[0:64, :], qf[64:128, :])
                nc.gpsimd.dma_start(qs[64:128, :], qf[0:64, :])
                nc.vector.tensor_mul(qf[:], qf[:], cos_sb[:, cs])
                nc.vector.tensor_mul(qs[:], qs[:], ssig_sb[:, cs])
                nc.vector.tensor_add(dst_ap, qf[:], qs[:])

            def proj_chunk(c):
                """Projections for T positions [c*512, (c+1)*512)."""
                cs = slice(c * 512, (c + 1) * 512)
                # k^T chunk
                kp = psum.tile([128, 512], f32, tag="proj", bufs=2)
                for k in range(NK):
                    nc.tensor.matmul(kp[:], wk[k][:], xT[k][:, cs],
                                     start=(k == 0), stop=(k == NK - 1))
                rope_to(kT[:, cs], kp, c)
                # q^T chunks, one per head
                for h in range(HEADS_PER_CORE):
                    qp = psum.tile([128, 512], f32, tag="proj", bufs=2)
                    for k in range(NK):
                        nc.tensor.matmul(qp[:], wq[k][:, ts(h, 128)],
                                         xT[k][:, cs],
                                         start=(k == 0), stop=(k == NK - 1))
                    rope_to(qT[:, h, cs], qp, c)
                # v tiles (natural layout), 4 per chunk
                for tt in range(4 * c, 4 * c + 4):
                    vp_full = psum.tile([128, 512], f32, tag="proj", bufs=2, name="vp")
                    vp = vp_full[:, :128]
                    for k in range(NK):
                        nc.tensor.matmul(vp[:], xT[k][:, ts(tt, 128)],
                                         wv[k][:],
                                         start=(k == 0), stop=(k == NK - 1))
                    nc.vector.tensor_copy(v_sb[:, tt, :], vp[:])

            def attn_chunk(c):
                """Attention for q chunk c (all 4 heads) -> attnT tile."""
                attn_t = attn_pool.tile([128, HEADS_PER_CORE, 512], bf16,
                                        tag="attnT")
                nj = 4 * c + 4
                for h in range(HEADS_PER_CORE):
                    out_ps = psum.tile([128, 512], f32, tag="out", bufs=2)
                    sums_ps = psum.tile([128, 512], f32, tag="sums", bufs=1)
                    for j in range(nj):
                        # columns < o*128 of this [tk-tile, q-chunk] block
                        # are fully masked (tk > tq): skip them everywhere
                        o = j - 4 * c
                        lo = max(o, 0) * 128
                        qs0 = c * 512 + lo
                        s_ps = psum.tile([128, 512], f32, tag="s", bufs=3)
                        nc.tensor.matmul(s_ps[:, lo:], kT[:, ts(j, 128)],
                                         qT[:, h, qs0:(c + 1) * 512],
                                         start=True, stop=True)
                        if o >= 0:
                            nc.vector.tensor_add(s_ps[:, lo:lo + 128],
                                                 s_ps[:, lo:lo + 128],
                                                 mask_sb[:])
                        p = p_pool.tile([128, 512], bf16, tag="p")
                        nc.scalar.activation(p[:, lo:], s_ps[:, lo:], Exp,
                                             bias=0.0, scale=SCALE)
                        nc.tensor.matmul(out_ps[:, lo:], v_sb[:, j, :],
                                         p[:, lo:],
                                         start=(j == 0), stop=(j == nj - 1))
                        nc.tensor.matmul(sums_ps[:, lo:], ones_sb[:],
                                         p[:, lo:],
                                         start=(j == 0), stop=(j == nj - 1))
                    rc = recip_pool.tile([128, 512], f32, tag="rc")
                    nc.vector.reciprocal_approx_fast(out=rc[:], in_=sums_ps[:])
                    nc.vector.tensor_mul(attn_t[:, h, :], out_ps[:], rc[:])
                return attn_t

            def wo_chunk(c, attn_t):
                """Output projection for q chunk c."""
                for tq in range(4):
                    row0 = (4 * c + tq) * 128
                    for nn in range(4):
                        yp = psum.tile([128, 512], f32, tag="s", bufs=3)
                        for h in range(HEADS_PER_CORE):
                            nc.tensor.matmul(yp[:],
                                             attn_t[:, h, ts(tq, 128)],
                                             wo[h][:, ts(nn, 512)],
                                             start=(h == 0), stop=(h == 3))
                        ysb = y_pool.tile([128, 512], f32, tag="y")
                        nc.any.tensor_copy(out=ysb[:], in_=yp[:])
                        nc.sync.dma_start(
                            y_d[row0:row0 + 128, ts(nn, 512)], ysb[:])

            # ---- emission order: interleave so attention/Wo of chunk c
            # overlap projections of chunk c+2 ----
            for c in range(NCQ):
                proj_chunk(c)
                at = attn_chunk(c)
                wo_chunk(c, at)

    nc.finalize()
    return nc


def _prep_inputs(x, Wq, Wk, Wv, Wo, t_len=T):
    """Host-side shard + layout prep -> per-core input maps."""
    import ml_dtypes
    bf16 = ml_dtypes.bfloat16

    x = np.asarray(x, np.float32)
    Wq = np.asarray(Wq, np.float32)
    Wk = np.asarray(Wk, np.float32)
    Wv = np.asarray(Wv, np.float32)
    Wo = np.asarray(Wo, np.float32)

    # RoPE de-interleave permutation within one head: [evens | odds]
    perm = np.concatenate([np.arange(0, HD, 2), np.arange(1, HD, 2)])

    # rope tables (match reference: freqs = t * base**(-2j/HD))
    inv = 1.0 / (ROPE_BASE ** (np.arange(0, HD, 2, dtype=np.float32) / HD))
    tpos = np.arange(t_len, dtype=np.float32)
    f = inv[:, None] * tpos[None, :]                       # [64, T]
    cos_dup = np.concatenate([np.cos(f), np.cos(f)], 0)    # [128, T]
    ssig = np.concatenate([-np.sin(f), np.sin(f)], 0)      # [128, T]
    cos_dup = cos_dup.astype(bf16)
    ssig = ssig.astype(bf16)

    # strict-lower-triangular causal mask template for the diagonal
    # [tk-tile, tq-tile] block (tk > tq within the 128x128 block)
    r = np.arange(128)[:, None]
    col = np.arange(128)[None, :]
    mask_t = np.where(r > col, MASK_VAL, 0.0).astype(bf16)

    in_maps = []
    for b in range(B):
        xT_b = np.ascontiguousarray(x[b, :t_len].T).astype(bf16)  # [D, T]
        for g in range(KV):
            wq_g = Wq[:, g * DQ:(g + 1) * DQ].reshape(D, HEADS_PER_CORE, HD)
            wq_g = np.ascontiguousarray(
                wq_g[:, :, perm].reshape(D, DQ)).astype(bf16)
            wk_g = np.ascontiguousarray(
                Wk[:, g * HD:(g + 1) * HD][:, perm]).astype(bf16)
            wv_g = np.ascontiguousarray(
                Wv[:, g * HD:(g + 1) * HD]).astype(bf16)
            wo_g = np.ascontiguousarray(
                Wo[g * DQ:(g + 1) * DQ, :]).astype(bf16)
            in_maps.append({
                "xT": xT_b, "wq": wq_g, "wk": wk_g, "wv": wv_g,
                "wo": wo_g, "cosd": cos_dup, "ssig": ssig, "mask": mask_t,
            })
    return in_maps


def run(inputs, trace=False, t_len=T):
    """Run the sharded kernel; returns (y_full, BassKernelResults)."""
    from concourse.bass_utils import run_bass_kernel_spmd

    key = ("nc", t_len)
    if key not in _CACHE:
        _CACHE[key] = _build_nc(t_len)
    nc = _CACHE[key]

    in_maps = _prep_inputs(inputs["x"], inputs["Wq"], inputs["Wk"],
                           inputs["Wv"], inputs["Wo"], t_len)
    res = run_bass_kernel_spmd(nc, in_maps, list(range(N_CORES)), trace=trace)

    y = np.empty((B, t_len, D), np.float32)
    for b in range(B):
        acc = np.zeros((t_len, D), np.float32)
        for g in range(KV):
            acc += np.asarray(res.results[b * KV + g]["y"], np.float32)
        y[b] = acc
    return y, res


def kernel(**inputs) -> np.ndarray:
    y, _ = run(inputs, trace=False)
    return y



# revision 2
# speedup vs baseline: 1.0414x; 1.0414x over previous
"""Tensor-parallel GQA multi-head-attention kernel for 8 trn2 NeuronCores.

Problem: B=2, T=2048, D=2048, H=16 q-heads, KV=4 kv-heads, HD=128,
causal attention with interleaved RoPE, y = attn_out @ Wo.

Sharding (tensor-parallel over heads, per the hint):
  core c = b*4 + g   (b = batch index, g = kv-head / q-head-group index)
  Each core computes q-heads 4g..4g+3 and kv-head g for batch b, plus the
  partial output  y_partial = attn_heads @ Wo[rows of those heads]  (row-
  parallel Wo).  The host sums the partials per batch (the unshard of the
  row-parallel all-reduce) and stacks the 2 batches.

v1 performance structure (vs the v0 baseline at ~320us):
  - inputs are host-packed chunk-contiguous so every load is ONE large DMA
    (128 partitions x 4-16KB contiguous), split across BOTH HWDGE queues
    (sync + scalar) in priority order -> input phase is bandwidth-bound,
    not descriptor-latency-bound.
  - ~9us of warmup matmuls on a zeroed tile overlap the input DMA and keep
    the PE HAM clock-gate at K=8/8 (2.4 GHz) from the start.
  - emission is slot-pipelined: attention blocks of chunk c are interleaved
    with projection matmuls of chunk c+1 and Wo matmuls of chunk c-1
    ("filler" MMs), and each block's PV/sums matmuls trail its S matmul by
    one block, so the PE never waits on the Scalar-engine exp
    ((N+352)/1.2ns latency) and the ACT load is spread over the whole slot.
  - Wo is computed in two head-pair passes writing separate bf16 outputs
    (y01 = heads 0,1; y23 = heads 2,3); pass 1 of the last chunk is issued
    mid-slot, so the end-of-kernel tail is only ~32 matmuls + stores.
    The host sums 4 bf16 partials per (batch, head-pair) in fp32.
"""

import math
import sys
from collections import defaultdict

import numpy as np

for _p in ("/opt/trn_rl_repo", "/root/.axon_site",
           "/root/.axon_site/_ro/trn_rl_repo",
           "/root/.axon_site/_ro/pypackages"):
    if _p not in sys.path:
        sys.path.append(_p)

B, T, D = 2, 2048, 2048
H, KV, HD = 16, 4, 128
ROPE_BASE = 10000.0
N_CORES = 8
HEADS_PER_CORE = 4
DQ = HEADS_PER_CORE * HD  # 512 q-dims per core
NK = D // 128             # contraction chunks for projections
SCALE = 1.0 / math.sqrt(HD)
MASK_VAL = -30000.0
N_WARM = 34

_CACHE = {}


class _Thunks:
    """A filler generator plus its remaining-yield count."""

    def __init__(self, gen, n):
        self.gen = gen
        self.n = n


def _build_nc(t_len=T):
    """Build the single-core SPMD Bass/Tile program (cached)."""
    import concourse.bass as bass
    import concourse.mybir as mybir
    import concourse.tile as tile
    from concourse import bacc

    f32 = mybir.dt.float32
    bf16 = mybir.dt.bfloat16
    ts = bass.ts

    NT = t_len // 128        # number of 128-row T tiles
    NCQ = t_len // 512       # number of 512-wide q chunks

    nc = bacc.Bacc("TRN2", target_bir_lowering=False, debug=False,
                   num_devices=N_CORES)

    xt_d = nc.dram_tensor("xt", [128, NCQ, NK, 512], bf16,
                          kind="ExternalInput").ap()
    wq_d = nc.dram_tensor("wq", [128, NK, DQ], bf16, kind="ExternalInput").ap()
    wk_d = nc.dram_tensor("wk", [128, NK, HD], bf16, kind="ExternalInput").ap()
    wv_d = nc.dram_tensor("wv", [128, NK, HD], bf16, kind="ExternalInput").ap()
    wo_d = nc.dram_tensor("wo", [128, HEADS_PER_CORE, D], bf16,
                          kind="ExternalInput").ap()
    cos_d = nc.dram_tensor("cosd", [128, t_len], bf16, kind="ExternalInput").ap()
    ssig_d = nc.dram_tensor("ssig", [128, t_len], bf16, kind="ExternalInput").ap()
    mask_d = nc.dram_tensor("mask", [128, 128], bf16, kind="ExternalInput").ap()
    y01_d = nc.dram_tensor("y01", [t_len, D], bf16, kind="ExternalOutput").ap()
    y23_d = nc.dram_tensor("y23", [t_len, D], bf16, kind="ExternalOutput").ap()

    Exp = mybir.ActivationFunctionType.Exp

    with tile.TileContext(nc) as tc:
        with (
            tc.tile_pool(name="const", bufs=1) as const,
            tc.tile_pool(name="qkv", bufs=1) as qkv,
            tc.tile_pool(name="attn", bufs=2) as attn_pool,
            tc.tile_pool(name="p", bufs=4) as p_pool,
            tc.tile_pool(name="rope", bufs=2) as rope_pool,
            tc.tile_pool(name="recip", bufs=2) as recip_pool,
            tc.tile_pool(name="y", bufs=3) as y_pool,
            tc.tile_pool(name="psum", bufs=1, space="PSUM") as psum,
        ):
            # ---- warmup source (zeros) + PE warmup matmuls: keep the HAM
            # clock-gate busy/warm while the real inputs stream in ----
            wz = const.tile([128, 512], bf16, tag="wz")
            nc.vector.memset(wz[:], 0.0)
            ones_sb = const.tile([128, 128], bf16, tag="ones")
            nc.vector.memset(ones_sb[:], 1.0)
            for _ in range(N_WARM):
                wp = psum.tile([128, 512], f32, tag="mm", bufs=3, name="wp")
                nc.tensor.matmul(wp[:], wz[:, 0:128], wz[:],
                                 start=True, stop=True)

            # ---- input loads: one large DMA per tensor/chunk, split across
            # the two HWDGE queues (sync + scalar) in priority order ----
            xt_sb = const.tile([128, NCQ, NK, 512], bf16, tag="xt")
            wq_sb = const.tile([128, NK, DQ], bf16, tag="wq")
            wk_sb = const.tile([128, NK, HD], bf16, tag="wk")
            wv_sb = const.tile([128, NK, HD], bf16, tag="wv")
            wo_sb = const.tile([128, HEADS_PER_CORE, D], bf16, tag="wo")
            cos_sb = const.tile([128, t_len], bf16, tag="cos")
            ssig_sb = const.tile([128, t_len], bf16, tag="ssig")
            mask_sb = const.tile([128, 128], bf16, tag="mask")

            nc.sync.dma_start(wk_sb[:], wk_d[:])
            nc.sync.dma_start(wv_sb[:], wv_d[:])
            for c in range(NCQ):
                nc.sync.dma_start(xt_sb[:, c], xt_d[:, c])
            nc.scalar.dma_start(wq_sb[:], wq_d[:])
            nc.scalar.dma_start(cos_sb[:], cos_d[:])
            nc.scalar.dma_start(ssig_sb[:], ssig_d[:])
            nc.scalar.dma_start(mask_sb[:], mask_d[:])
            nc.scalar.dma_start(wo_sb[:], wo_d[:])

            # persistent activations
            qT = qkv.tile([128, HEADS_PER_CORE, t_len], bf16, tag="qT")
            kT = qkv.tile([128, t_len], bf16, tag="kT")
            v_sb = qkv.tile([128, NT, HD], bf16, tag="v")

            def rope_to(dst_ap, psum_tile, c):
                """Apply RoPE to a [128, 512] psum tile (rows = [even|odd]
                dims of one head, cols = T positions of chunk c); write bf16
                to dst_ap."""
                cs = slice(c * 512, (c + 1) * 512)
                qf = rope_pool.tile([128, 512], f32, tag="qf")
                nc.vector.tensor_copy(qf[:], psum_tile[:])
                qs = rope_pool.tile([128, 512], f32, tag="qs")
                nc.gpsimd.dma_start(qs[0:64, :], qf[64:128, :])
                nc.gpsimd.dma_start(qs[64:128, :], qf[0:64, :])
                nc.vector.tensor_mul(qf[:], qf[:], cos_sb[:, cs])
                nc.vector.tensor_mul(qs[:], qs[:], ssig_sb[:, cs])
                nc.vector.tensor_add(dst_ap, qf[:], qs[:])

            def gen_proj(c):
                """Projection matmuls for chunk c, one yield per matmul."""
                cs = slice(c * 512, (c + 1) * 512)
                kp = psum.tile([128, 512], f32, tag="acc", bufs=2, name="kp")
                for k in range(NK):
                    nc.tensor.matmul(kp[:], wk_sb[:, k, :], xt_sb[:, c, k, :],
                                     start=(k == 0), stop=(k == NK - 1))
                    if k < NK - 1:
                        yield
                rope_to(kT[:, cs], kp, c)
                yield
                for h in range(HEADS_PER_CORE):
                    qp = psum.tile([128, 512], f32, tag="acc", bufs=2,
                                   name="qp")
                    for k in range(NK):
                        nc.tensor.matmul(qp[:], wq_sb[:, k, ts(h, 128)],
                                         xt_sb[:, c, k, :],
                                         start=(k == 0), stop=(k == NK - 1))
                        if k < NK - 1:
                            yield
                    rope_to(qT[:, h, cs], qp, c)
                    yield
                for tl in range(4):
                    vp = psum.tile([128, 512], f32, tag="acc", bufs=2,
                                   name="vp")
                    for k in range(NK):
                        nc.tensor.matmul(vp[:, 0:128],
                                         xt_sb[:, c, k, ts(tl, 128)],
                                         wv_sb[:, k, :],
                                         start=(k == 0), stop=(k == NK - 1))
                        if k < NK - 1:
                            yield
                    nc.vector.tensor_copy(v_sb[:, 4 * c + tl, :], vp[:, 0:128])
                    yield

            PROJ_YIELDS = NK * (1 + HEADS_PER_CORE + 4)

            def gen_wo(c, attn_t, phase):
                """Output-projection matmuls for chunk c, head pair `phase`
                (0 -> heads 0,1 -> y01; 1 -> heads 2,3 -> y23)."""
                h0, h1 = (0, 1) if phase == 0 else (2, 3)
                yd = y01_d if phase == 0 else y23_d
                for nn in range(4):
                    for tq in range(4):
                        yp = psum.tile([128, 512], f32, tag="acc", bufs=2,
                                       name="yp")
                        nc.tensor.matmul(yp[:], attn_t[:, h0, ts(tq, 128)],
                                         wo_sb[:, h0, ts(nn, 512)],
                                         start=True, stop=False)
                        yield
                        nc.tensor.matmul(yp[:], attn_t[:, h1, ts(tq, 128)],
                                         wo_sb[:, h1, ts(nn, 512)],
                                         start=False, stop=True)
                        row0 = (4 * c + tq) * 128
                        ysb = y_pool.tile([128, 512], bf16, tag="y")
                        nc.vector.tensor_copy(ysb[:], yp[:])
                        nc.scalar.dma_start(yd[row0:row0 + 128, ts(nn, 512)],
                                            ysb[:])
                        yield

            WO_YIELDS = 32

            def emit_attn_slot(c, attn_t, head_fillers):
                """Attention for chunk c (4 heads), with PV/sums trailing one
                block behind S/exp and filler matmuls pumped in between."""
                nj = 4 * c + 4
                nb = nj * HEADS_PER_CORE
                avail = []
                state = {"rem": 0, "rr": 0}

                def pump(kmax):
                    done = 0
                    while done < kmax and avail:
                        idx = state["rr"] % len(avail)
                        it = avail[idx]
                        try:
                            next(it.gen)
                            it.n -= 1
                            state["rem"] -= 1
                            done += 1
                            state["rr"] += 1
                        except StopIteration:
                            avail.pop(idx)
                    return done

                def emit_pv(pend):
                    (h, j, p, lo, out_ps, sums_ps, last) = pend
                    nc.tensor.matmul(out_ps[:, lo:], v_sb[:, j, :], p[:, lo:],
                                     start=(j == 0), stop=last)
                    nc.tensor.matmul(sums_ps[:, lo:], ones_sb[:], p[:, lo:],
                                     start=(j == 0), stop=last)
                    if last:
                        rc = recip_pool.tile([128, 512], f32, tag="rc")
                        nc.vector.reciprocal_approx_fast(out=rc[:],
                                                         in_=sums_ps[:])
                        nc.vector.tensor_mul(attn_t[:, h, :], out_ps[:], rc[:])

                pend = None
                bi = 0
                for h in range(HEADS_PER_CORE):
                    for f in head_fillers.get(h, ()):
                        avail.append(f)
                        state["rem"] += f.n
                    out_ps = psum.tile([128, 512], f32, tag="out", bufs=2,
                                       name="out")
                    sums_ps = psum.tile([128, 512], f32, tag="sums", bufs=1,
                                        name="sums")
                    for j in range(nj):
                        o = j - 4 * c
                        lo = max(o, 0) * 128
                        qs0 = c * 512 + lo
                        s_ps = psum.tile([128, 512], f32, tag="mm", bufs=3,
                                         name="s")
                        nc.tensor.matmul(s_ps[:, lo:], kT[:, ts(j, 128)],
                                         qT[:, h, qs0:(c + 1) * 512],
                                         start=True, stop=True)
                        if o >= 0:
                            nc.vector.tensor_add(s_ps[:, lo:lo + 128],
                                                 s_ps[:, lo:lo + 128],
                                                 mask_sb[:])
                        p = p_pool.tile([128, 512], bf16, tag="p")
                        nc.scalar.activation(p[:, lo:], s_ps[:, lo:], Exp,
                                             bias=0.0, scale=SCALE)
                        blocks_left = nb - bi
                        k = min(3, max(1, state["rem"] // blocks_left))
                        pump(k)
                        if pend is not None:
                            emit_pv(pend)
                        pend = (h, j, p, lo, out_ps, sums_ps, j == nj - 1)
                        bi += 1
                emit_pv(pend)
                pump(10 ** 9)

            # ---- emission: proj(0) dense, then pipelined slots ----
            for _ in gen_proj(0):
                pass

            attn_ts = {}
            for c in range(NCQ):
                attn_ts[c] = attn_pool.tile([128, HEADS_PER_CORE, 512], bf16,
                                            tag="attnT", name=f"attnT{c}")

            for c in range(NCQ):
                hf = defaultdict(list)
                if c + 1 < NCQ:
                    hf[0].append(_Thunks(gen_proj(c + 1), PROJ_YIELDS))
                if c >= 1:
                    hf[0].append(_Thunks(gen_wo(c - 1, attn_ts[c - 1], 0),
                                         WO_YIELDS))
                    hf[0].append(_Thunks(gen_wo(c - 1, attn_ts[c - 1], 1),
                                         WO_YIELDS))
                if c == NCQ - 1:
                    # last chunk: heads 0,1 finish mid-slot, so their Wo pass
                    # can fill the tail of the slot
                    hf[2].append(_Thunks(gen_wo(c, attn_ts[c], 0), WO_YIELDS))
                emit_attn_slot(c, attn_ts[c], hf)

            # final tail: heads 2,3 of the last chunk
            for _ in gen_wo(NCQ - 1, attn_ts[NCQ - 1], 1):
                pass

    nc.finalize()
    return nc


def _prep_inputs(x, Wq, Wk, Wv, Wo, t_len=T):
    """Host-side shard + layout prep -> per-core input maps."""
    import ml_dtypes
    bf16 = ml_dtypes.bfloat16

    NCQ = t_len // 512

    x = np.asarray(x, np.float32)
    Wq = np.asarray(Wq, np.float32)
    Wk = np.asarray(Wk, np.float32)
    Wv = np.asarray(Wv, np.float32)
    Wo = np.asarray(Wo, np.float32)

    # RoPE de-interleave permutation within one head: [evens | odds]
    perm = np.concatenate([np.arange(0, HD, 2), np.arange(1, HD, 2)])

    # rope tables (match reference: freqs = t * base**(-2j/HD))
    inv = 1.0 / (ROPE_BASE ** (np.arange(0, HD, 2, dtype=np.float32) / HD))
    tpos = np.arange(t_len, dtype=np.float32)
    f = inv[:, None] * tpos[None, :]                       # [64, T]
    cos_dup = np.concatenate([np.cos(f), np.cos(f)], 0).astype(bf16)
    ssig = np.concatenate([-np.sin(f), np.sin(f)], 0).astype(bf16)

    # strict-lower-triangular causal mask template for the diagonal
    # [tk-tile, tq-tile] block (tk > tq within the 128x128 block)
    r = np.arange(128)[:, None]
    col = np.arange(128)[None, :]
    mask_t = np.where(r > col, MASK_VAL, 0.0).astype(bf16)

    # chunk-contiguous xT packing: [128, NCQ, NK, 512]
    xt_b = []
    for b in range(B):
        a = x[b, :t_len].T.reshape(NK, 128, NCQ, 512).transpose(1, 2, 0, 3)
        xt_b.append(np.ascontiguousarray(a).astype(bf16))

    in_maps = []
    for b in range(B):
        for g in range(KV):
            wq_g = Wq[:, g * DQ:(g + 1) * DQ].reshape(D, HEADS_PER_CORE, HD)
            wq_g = wq_g[:, :, perm].reshape(NK, 128, DQ).transpose(1, 0, 2)
            wk_g = Wk[:, g * HD:(g + 1) * HD][:, perm]
            wk_g = wk_g.reshape(NK, 128, HD).transpose(1, 0, 2)
            wv_g = Wv[:, g * HD:(g + 1) * HD]
            wv_g = wv_g.reshape(NK, 128, HD).transpose(1, 0, 2)
            wo_g = Wo[g * DQ:(g + 1) * DQ, :]
            wo_g = wo_g.reshape(HEADS_PER_CORE, 128, D).transpose(1, 0, 2)
            in_maps.append({
                "xt": xt_b[b],
                "wq": np.ascontiguousarray(wq_g).astype(bf16),
                "wk": np.ascontiguousarray(wk_g).astype(bf16),
                "wv": np.ascontiguousarray(wv_g).astype(bf16),
                "wo": np.ascontiguousarray(wo_g).astype(bf16),
                "cosd": cos_dup, "ssig": ssig, "mask": mask_t,
            })
    return in_maps


def run(inputs, trace=False, t_len=T):
    """Run the sharded kernel; returns (y_full, BassKernelResults)."""
    from concourse.bass_utils import run_bass_kernel_spmd

    key = ("nc", t_len)
    if key not in _CACHE:
        _CACHE[key] = _build_nc(t_len)
    nc = _CACHE[key]

    in_maps = _prep_inputs(inputs["x"], inputs["Wq"], inputs["Wk"],
                           inputs["Wv"], inputs["Wo"], t_len)
    res = run_bass_kernel_spmd(nc, in_maps, list(range(N_CORES)), trace=trace)

    y = np.empty((B, t_len, D), np.float32)
    for b in range(B):
        acc = np.zeros((t_len, D), np.float32)
        for g in range(KV):
            r = res.results[b * KV + g]
            acc += np.asarray(r["y01"], np.float32)
            acc += np.asarray(r["y23"], np.float32)
        y[b] = acc
    return y, res


def kernel(**inputs) -> np.ndarray:
    y, _ = run(inputs, trace=False)
    return y


# revision 4
# speedup vs baseline: 1.0742x; 1.0314x over previous
"""Tensor-parallel GQA multi-head-attention kernel for 8 trn2 NeuronCores.

Problem: B=2, T=2048, D=2048, H=16 q-heads, KV=4 kv-heads, HD=128,
causal attention with interleaved RoPE, y = attn_out @ Wo.

Sharding (tensor-parallel over heads, per the hint):
  core c = b*4 + g   (b = batch index, g = kv-head / q-head-group index)
  Each core computes q-heads 4g..4g+3 and kv-head g for batch b, plus the
  partial output  y_partial = attn_heads @ Wo[rows of those heads]  (row-
  parallel Wo).  The host sums the partials per batch (the unshard of the
  row-parallel all-reduce) and stacks the 2 batches.

v1 performance structure (vs the v0 baseline at ~320us):
  - inputs are host-packed chunk-contiguous so every load is ONE large DMA
    (128 partitions x 4-16KB contiguous), split across BOTH HWDGE queues
    (sync + scalar) in priority order -> input phase is bandwidth-bound,
    not descriptor-latency-bound.
  - ~9us of warmup matmuls on a zeroed tile overlap the input DMA and keep
    the PE HAM clock-gate at K=8/8 (2.4 GHz) from the start.
  - emission is slot-pipelined: attention blocks of chunk c are interleaved
    with projection matmuls of chunk c+1 and Wo matmuls of chunk c-1
    ("filler" MMs), and each block's PV/sums matmuls trail its S matmul by
    one block, so the PE never waits on the Scalar-engine exp
    ((N+352)/1.2ns latency) and the ACT load is spread over the whole slot.
  - Wo is computed in two head-pair passes writing separate bf16 outputs
    (y01 = heads 0,1; y23 = heads 2,3); pass 1 of the last chunk is issued
    mid-slot, so the end-of-kernel tail is only ~32 matmuls + stores.
    The host sums 4 bf16 partials per (batch, head-pair) in fp32.
"""

import math
import sys
from collections import defaultdict

import numpy as np

for _p in ("/opt/trn_rl_repo", "/root/.axon_site",
           "/root/.axon_site/_ro/trn_rl_repo",
           "/root/.axon_site/_ro/pypackages"):
    if _p not in sys.path:
        sys.path.append(_p)

B, T, D = 2, 2048, 2048
H, KV, HD = 16, 4, 128
ROPE_BASE = 10000.0
N_CORES = 8
HEADS_PER_CORE = 4
DQ = HEADS_PER_CORE * HD  # 512 q-dims per core
NK = D // 128             # contraction chunks for projections
SCALE = 1.0 / math.sqrt(HD)
MASK_VAL = -30000.0
N_WARM = 28

_CACHE = {}


class _Thunks:
    """A filler generator plus its remaining-yield count."""

    def __init__(self, gen, n):
        self.gen = gen
        self.n = n


def _build_nc(t_len=T):
    """Build the single-core SPMD Bass/Tile program (cached)."""
    import concourse.bass as bass
    import concourse.mybir as mybir
    import concourse.tile as tile
    from concourse import bacc

    f32 = mybir.dt.float32
    bf16 = mybir.dt.bfloat16
    ts = bass.ts

    NT = t_len // 128        # number of 128-row T tiles
    NCQ = t_len // 512       # number of 512-wide q chunks

    nc = bacc.Bacc("TRN2", target_bir_lowering=False, debug=False,
                   num_devices=N_CORES)

    xt_d = nc.dram_tensor("xt", [128, NCQ, NK, 512], bf16,
                          kind="ExternalInput").ap()
    wq_d = nc.dram_tensor("wq", [128, NK, DQ], bf16, kind="ExternalInput").ap()
    wk_d = nc.dram_tensor("wk", [128, NK, HD], bf16, kind="ExternalInput").ap()
    wv_d = nc.dram_tensor("wv", [128, NK, HD], bf16, kind="ExternalInput").ap()
    wo_d = nc.dram_tensor("wo", [128, HEADS_PER_CORE, D], bf16,
                          kind="ExternalInput").ap()
    cos_d = nc.dram_tensor("cosd", [128, t_len], bf16, kind="ExternalInput").ap()
    ssig_d = nc.dram_tensor("ssig", [128, t_len], bf16, kind="ExternalInput").ap()
    mask_d = nc.dram_tensor("mask", [128, 128], bf16, kind="ExternalInput").ap()
    y01_d = nc.dram_tensor("y01", [t_len, D], bf16, kind="ExternalOutput").ap()
    y23_d = nc.dram_tensor("y23", [t_len, D], bf16, kind="ExternalOutput").ap()

    Exp = mybir.ActivationFunctionType.Exp

    with tile.TileContext(nc) as tc:
        with (
            tc.tile_pool(name="const", bufs=1) as const,
            tc.tile_pool(name="qkv", bufs=1) as qkv,
            tc.tile_pool(name="attn", bufs=2) as attn_pool,
            tc.tile_pool(name="p", bufs=4) as p_pool,
            tc.tile_pool(name="rope", bufs=2) as rope_pool,
            tc.tile_pool(name="recip", bufs=2) as recip_pool,
            tc.tile_pool(name="y", bufs=3) as y_pool,
            tc.tile_pool(name="psum", bufs=1, space="PSUM") as psum,
        ):
            # ---- warmup source (zeros) + PE warmup matmuls: keep the HAM
            # clock-gate busy/warm while the real inputs stream in ----
            wz = const.tile([128, 512], bf16, tag="wz")
            nc.vector.memset(wz[:], 0.0)
            ones_sb = const.tile([128, 128], bf16, tag="ones")
            nc.vector.memset(ones_sb[:], 1.0)
            for _ in range(N_WARM):
                wp = psum.tile([128, 512], f32, tag="mm", bufs=3, name="wp")
                nc.tensor.matmul(wp[:], wz[:, 0:128], wz[:],
                                 start=True, stop=True)

            # ---- input loads: one large DMA per tensor/chunk, split across
            # the two HWDGE queues (sync + scalar) in priority order ----
            xt_sb = const.tile([128, NCQ, NK, 512], bf16, tag="xt")
            wq_sb = const.tile([128, NK, DQ], bf16, tag="wq")
            wk_sb = const.tile([128, NK, HD], bf16, tag="wk")
            wv_sb = const.tile([128, NK, HD], bf16, tag="wv")
            wo_sb = const.tile([128, HEADS_PER_CORE, D], bf16, tag="wo")
            cos_sb = const.tile([128, t_len], bf16, tag="cos")
            ssig_sb = const.tile([128, t_len], bf16, tag="ssig")
            mask_sb = const.tile([128, 128], bf16, tag="mask")

            nc.sync.dma_start(xt_sb[:, 0], xt_d[:, 0])
            nc.sync.dma_start(xt_sb[:, 1], xt_d[:, 1])
            nc.scalar.dma_start(wk_sb[:], wk_d[:])
            nc.scalar.dma_start(wv_sb[:], wv_d[:])
            nc.scalar.dma_start(cos_sb[:], cos_d[:])
            nc.scalar.dma_start(ssig_sb[:], ssig_d[:])
            nc.scalar.dma_start(mask_sb[:], mask_d[:])
            nc.scalar.dma_start(wq_sb[:], wq_d[:])
            for c in range(2, NCQ):
                nc.scalar.dma_start(xt_sb[:, c], xt_d[:, c])
            nc.scalar.dma_start(wo_sb[:], wo_d[:])

            # persistent activations
            qT = qkv.tile([128, HEADS_PER_CORE, t_len], bf16, tag="qT")
            kT = qkv.tile([128, t_len], bf16, tag="kT")
            v_sb = qkv.tile([128, NT, HD], bf16, tag="v")

            def rope_to(dst_ap, psum_tile, c):
                """Apply RoPE to a [128, 512] psum tile (rows = [even|odd]
                dims of one head, cols = T positions of chunk c); write bf16
                to dst_ap."""
                cs = slice(c * 512, (c + 1) * 512)
                qf = rope_pool.tile([128, 512], f32, tag="qf")
                nc.vector.tensor_copy(qf[:], psum_tile[:])
                qs = rope_pool.tile([128, 512], f32, tag="qs")
                nc.sync.dma_start(qs[0:64, :], qf[64:128, :])
                nc.sync.dma_start(qs[64:128, :], qf[0:64, :])
                nc.gpsimd.tensor_mul(qf[:], qf[:], cos_sb[:, cs])
                nc.gpsimd.tensor_mul(qs[:], qs[:], ssig_sb[:, cs])
                nc.gpsimd.tensor_add(dst_ap, qf[:], qs[:])

            def gen_proj(c):
                """Projection matmuls for chunk c, one yield per matmul."""
                cs = slice(c * 512, (c + 1) * 512)
                kp = psum.tile([128, 512], f32, tag="acc", bufs=2, name="kp")
                for k in range(NK):
                    nc.tensor.matmul(kp[:], wk_sb[:, k, :], xt_sb[:, c, k, :],
                                     start=(k == 0), stop=(k == NK - 1))
                    if k < NK - 1:
                        yield
                rope_to(kT[:, cs], kp, c)
                yield
                for tl in range(4):
                    vp = psum.tile([128, 512], f32, tag="acc", bufs=2,
                                   name="vp")
                    for k in range(NK):
                        nc.tensor.matmul(vp[:, 0:128],
                                         xt_sb[:, c, k, ts(tl, 128)],
                                         wv_sb[:, k, :],
                                         start=(k == 0), stop=(k == NK - 1))
                        if k < NK - 1:
                            yield
                    nc.vector.tensor_copy(v_sb[:, 4 * c + tl, :], vp[:, 0:128])
                    yield
                for h in range(HEADS_PER_CORE):
                    qp = psum.tile([128, 512], f32, tag="acc", bufs=2,
                                   name="qp")
                    for k in range(NK):
                        nc.tensor.matmul(qp[:], wq_sb[:, k, ts(h, 128)],
                                         xt_sb[:, c, k, :],
                                         start=(k == 0), stop=(k == NK - 1))
                        if k < NK - 1:
                            yield
                    rope_to(qT[:, h, cs], qp, c)
                    yield

            PROJ_YIELDS = NK * (1 + HEADS_PER_CORE + 4)

            def gen_wo(c, attn_t, phase, tag="acc", bufs=2, alt=False):
                """Output-projection matmuls for chunk c, head pair `phase`
                (0 -> heads 0,1 -> y01; 1 -> heads 2,3 -> y23)."""
                h0, h1 = (0, 1) if phase == 0 else (2, 3)
                yd = y01_d if phase == 0 else y23_d
                for gi, (nn, tq) in enumerate(
                        (n, t) for n in range(4) for t in range(4)):
                    yp = psum.tile([128, 512], f32, tag=tag, bufs=bufs,
                                   name="yp")
                    nc.tensor.matmul(yp[:], attn_t[:, h0, ts(tq, 128)],
                                     wo_sb[:, h0, ts(nn, 512)],
                                     start=True, stop=False)
                    yield
                    nc.tensor.matmul(yp[:], attn_t[:, h1, ts(tq, 128)],
                                     wo_sb[:, h1, ts(nn, 512)],
                                     start=False, stop=True)
                    row0 = (4 * c + tq) * 128
                    ysb = y_pool.tile([128, 512], bf16, tag="y")
                    if alt and gi % 2:
                        nc.scalar.copy(ysb[:], yp[:])
                    else:
                        nc.vector.tensor_copy(ysb[:], yp[:])
                    nc.sync.dma_start(yd[row0:row0 + 128, ts(nn, 512)],
                                      ysb[:])
                    yield

            WO_YIELDS = 32

            def emit_attn_slot(c, attn_t, head_fillers):
                """Attention for chunk c (4 heads), with PV/sums trailing one
                block behind S/exp and filler matmuls pumped in between."""
                nj = 4 * c + 4
                nb = nj * HEADS_PER_CORE
                avail = []
                state = {"rem": 0, "rr": 0}

                def pump(kmax):
                    done = 0
                    while done < kmax and avail:
                        idx = state["rr"] % len(avail)
                        it = avail[idx]
                        try:
                            next(it.gen)
                            it.n -= 1
                            state["rem"] -= 1
                            done += 1
                            state["rr"] += 1
                        except StopIteration:
                            avail.pop(idx)
                    return done

                def emit_pv(pend):
                    (h, j, p, lo, out_ps, sums_ps, last) = pend
                    nc.tensor.matmul(out_ps[:, lo:], v_sb[:, j, :], p[:, lo:],
                                     start=(j == 0), stop=last)
                    nc.tensor.matmul(sums_ps[:, lo:], ones_sb[:], p[:, lo:],
                                     start=(j == 0), stop=last)
                    if last:
                        rc = recip_pool.tile([128, 512], f32, tag="rc")
                        nc.vector.reciprocal_approx_fast(out=rc[:],
                                                         in_=sums_ps[:])
                        nc.vector.tensor_mul(attn_t[:, h, :], out_ps[:], rc[:])

                pend = None
                bi = 0
                for h in range(HEADS_PER_CORE):
                    for f in head_fillers.get(h, ()):
                        avail.append(f)
                        state["rem"] += f.n
                    out_ps = psum.tile([128, 512], f32, tag="out", bufs=2,
                                       name="out")
                    sums_ps = psum.tile([128, 512], f32, tag="sums", bufs=1,
                                        name="sums")
                    for j in range(nj):
                        o = j - 4 * c
                        lo = max(o, 0) * 128
                        qs0 = c * 512 + lo
                        s_ps = psum.tile([128, 512], f32, tag="mm", bufs=3,
                                         name="s")
                        nc.tensor.matmul(s_ps[:, lo:], kT[:, ts(j, 128)],
                                         qT[:, h, qs0:(c + 1) * 512],
                                         start=True, stop=True)
                        if o >= 0:
                            nc.vector.tensor_add(s_ps[:, lo:lo + 128],
                                                 s_ps[:, lo:lo + 128],
                                                 mask_sb[:])
                        p = p_pool.tile([128, 512], bf16, tag="p")
                        nc.scalar.activation(p[:, lo:], s_ps[:, lo:], Exp,
                                             bias=0.0, scale=SCALE)
                        blocks_left = nb - bi
                        k = min(3, max(1, state["rem"] // blocks_left))
                        pump(k)
                        if pend is not None:
                            emit_pv(pend)
                        pend = (h, j, p, lo, out_ps, sums_ps, j == nj - 1)
                        bi += 1
                emit_pv(pend)
                pump(10 ** 9)

            # ---- emission: proj(0) dense, then pipelined slots ----
            for _ in gen_proj(0):
                pass

            attn_ts = {}
            for c in range(NCQ):
                attn_ts[c] = attn_pool.tile([128, HEADS_PER_CORE, 512], bf16,
                                            tag="attnT", name=f"attnT{c}")

            for c in range(NCQ):
                hf = defaultdict(list)
                if c + 1 < NCQ:
                    hf[0].append(_Thunks(gen_proj(c + 1), PROJ_YIELDS))
                if c >= 1:
                    hf[0].append(_Thunks(gen_wo(c - 1, attn_ts[c - 1], 0),
                                         WO_YIELDS))
                    hf[0].append(_Thunks(gen_wo(c - 1, attn_ts[c - 1], 1),
                                         WO_YIELDS))
                if c == NCQ - 1:
                    # last chunk: heads 0,1 finish mid-slot, so their Wo pass
                    # can fill the tail of the slot
                    hf[2].append(_Thunks(gen_wo(c, attn_ts[c], 0), WO_YIELDS))
                emit_attn_slot(c, attn_ts[c], hf)

            # final tail: heads 2,3 of the last chunk (mm psum ring is free
            # by now; alternate the psum->sbuf copies across DVE/ACT)
            for _ in gen_wo(NCQ - 1, attn_ts[NCQ - 1], 1, tag="mm", bufs=3,
                            alt=True):
                pass

    nc.finalize()
    return nc


def _prep_inputs(x, Wq, Wk, Wv, Wo, t_len=T):
    """Host-side shard + layout prep -> per-core input maps."""
    import ml_dtypes
    bf16 = ml_dtypes.bfloat16

    NCQ = t_len // 512

    x = np.asarray(x, np.float32)
    Wq = np.asarray(Wq, np.float32)
    Wk = np.asarray(Wk, np.float32)
    Wv = np.asarray(Wv, np.float32)
    Wo = np.asarray(Wo, np.float32)

    # RoPE de-interleave permutation within one head: [evens | odds]
    perm = np.concatenate([np.arange(0, HD, 2), np.arange(1, HD, 2)])

    # rope tables (match reference: freqs = t * base**(-2j/HD))
    inv = 1.0 / (ROPE_BASE ** (np.arange(0, HD, 2, dtype=np.float32) / HD))
    tpos = np.arange(t_len, dtype=np.float32)
    f = inv[:, None] * tpos[None, :]                       # [64, T]
    cos_dup = np.concatenate([np.cos(f), np.cos(f)], 0).astype(bf16)
    ssig = np.concatenate([-np.sin(f), np.sin(f)], 0).astype(bf16)

    # strict-lower-triangular causal mask template for the diagonal
    # [tk-tile, tq-tile] block (tk > tq within the 128x128 block)
    r = np.arange(128)[:, None]
    col = np.arange(128)[None, :]
    mask_t = np.where(r > col, MASK_VAL, 0.0).astype(bf16)

    # chunk-contiguous xT packing: [128, NCQ, NK, 512]
    xt_b = []
    for b in range(B):
        a = x[b, :t_len].T.reshape(NK, 128, NCQ, 512).transpose(1, 2, 0, 3)
        xt_b.append(np.ascontiguousarray(a).astype(bf16))

    in_maps = []
    for b in range(B):
        for g in range(KV):
            wq_g = Wq[:, g * DQ:(g + 1) * DQ].reshape(D, HEADS_PER_CORE, HD)
            wq_g = wq_g[:, :, perm].reshape(NK, 128, DQ).transpose(1, 0, 2)
            wk_g = Wk[:, g * HD:(g + 1) * HD][:, perm]
            wk_g = wk_g.reshape(NK, 128, HD).transpose(1, 0, 2)
            wv_g = Wv[:, g * HD:(g + 1) * HD]
            wv_g = wv_g.reshape(NK, 128, HD).transpose(1, 0, 2)
            wo_g = Wo[g * DQ:(g + 1) * DQ, :]
            wo_g = wo_g.reshape(HEADS_PER_CORE, 128, D).transpose(1, 0, 2)
            in_maps.append({
                "xt": xt_b[b],
                "wq": np.ascontiguousarray(wq_g).astype(bf16),
                "wk": np.ascontiguousarray(wk_g).astype(bf16),
                "wv": np.ascontiguousarray(wv_g).astype(bf16),
                "wo": np.ascontiguousarray(wo_g).astype(bf16),
                "cosd": cos_dup, "ssig": ssig, "mask": mask_t,
            })
    return in_maps


def run(inputs, trace=False, t_len=T):
    """Run the sharded kernel; returns (y_full, BassKernelResults)."""
    from concourse.bass_utils import run_bass_kernel_spmd

    key = ("nc", t_len)
    if key not in _CACHE:
        _CACHE[key] = _build_nc(t_len)
    nc = _CACHE[key]

    in_maps = _prep_inputs(inputs["x"], inputs["Wq"], inputs["Wk"],
                           inputs["Wv"], inputs["Wo"], t_len)
    res = run_bass_kernel_spmd(nc, in_maps, list(range(N_CORES)), trace=trace)

    y = np.empty((B, t_len, D), np.float32)
    for b in range(B):
        acc = np.zeros((t_len, D), np.float32)
        for g in range(KV):
            r = res.results[b * KV + g]
            acc += np.asarray(r["y01"], np.float32)
            acc += np.asarray(r["y23"], np.float32)
        y[b] = acc
    return y, res


def kernel(**inputs) -> np.ndarray:
    y, _ = run(inputs, trace=False)
    return y
